# revision 24
# baseline (speedup 1.0000x reference)
"""Trainium2 Bass kernel for nn_CnUpdateLayer (segment_reduce / LDPC check-node update).

reference: out[b, i] = prod_{j : mask[i,j]==1} x[b, j]   (x ~ N(0,1), never exactly 0)

Log-domain trick turns the masked product into one dense matmul pass:
    S[b,i] = sum_j mask[i,j] * ln(x[b,j]^2)        -> magnitude = exp(0.5*S)
    C[b,i] = sum_j mask[i,j] * (x[b,j] < 0)        -> sign = 1 - 2*(C mod 2)
Both contractions share the stationary mask operand, so each K-tile runs as ONE
TensorEngine matmul with the moving operand [ln(x^2)^T | neg^T] (N=256,
float32r: full-rate fp32 matmul, PSUM accumulates in fp32).

Sharding: OUT columns (mask rows) x 8 cores, no collectives.
Each core gets x^T (replicated) and its mask shard pre-transposed on host so
the contraction dim (IN) lands on SBUF partitions with no on-device transpose.

Raw Bass (no Tile framework): hand-placed semaphores, minimal instruction
count, DMAs issued from different engines' HWDGE queues for parallelism.

Hardcoded problem shape: x [128, 1024] f32, layer_mask [1024, 1024] f32.
"""

import sys
from contextlib import ExitStack

import numpy as np

sys.path.insert(0, "/opt/trn_rl_repo")

import concourse.bacc as bacc
import concourse.bass as bass
from concourse import mybir
from concourse.bass_utils import run_bass_kernel_spmd

B = 128          # batch
IN = 1024        # in_features (contraction dim)
OUT = 1024       # out_features
NCORES = 8
O_SHARD = OUT // NCORES   # 128 mask rows per core
NK = IN // 128            # 8 K-tiles

F32 = mybir.dt.float32
F32R = mybir.dt.float32r
I32 = mybir.dt.int32
AF = mybir.ActivationFunctionType
ALU = mybir.AluOpType

# act_func_sets[6] = natural_log_exp_and_others: serves ln, exp, square, copy
ACT_TABLE_LN_EXP = 6


def build_nc():
    nc = bacc.Bacc(None, target_bir_lowering=False)
    # Host pre-packs the exact SBUF image: row p holds [xT[k*128+p, :] for k in NK]
    xt = nc.declare_dram_parameter("xt", [128, NK * B], F32, isOutput=False)
    mt = nc.declare_dram_parameter("mt", [128, NK * O_SHARD], F32R, isOutput=False)
    out = nc.declare_dram_parameter("out", [O_SHARD, B], F32, isOutput=True)

    xt3 = xt[:, :].rearrange("p (k b) -> p k b", k=NK)       # [128, NK, B]
    mt3 = mt[:, :].rearrange("p (k o) -> p k o", k=NK)       # [128, NK, O]

    with ExitStack() as ctx:
        xs = ctx.enter_context(nc.sbuf_tensor([128, NK, B], F32))       # x^T chunks
        ms = ctx.enter_context(nc.sbuf_tensor([128, NK, O_SHARD], F32R))
        sq = ctx.enter_context(nc.sbuf_tensor([128, NK, B], F32))       # x^2
        ln = ctx.enter_context(nc.sbuf_tensor([128, NK, 2 * B], F32R))  # [ln(x^2) | neg]
        ps = ctx.enter_context(nc.psum_tensor([128, 2 * B], F32))
        mag = ctx.enter_context(nc.sbuf_tensor([128, B], F32))
        ci = ctx.enter_context(nc.sbuf_tensor([128, B], I32))
        res = ctx.enter_context(nc.sbuf_tensor([128, B], F32))

        dma_x = ctx.enter_context(nc.semaphore("dma_x"))
        dma_m = ctx.enter_context(nc.semaphore("dma_m"))
        dma_o = ctx.enter_context(nc.semaphore("dma_o"))
        s_sq = ctx.enter_context(nc.semaphore("s_sq"))
        s_ln = ctx.enter_context(nc.semaphore("s_ln"))
        s_neg = ctx.enter_context(nc.semaphore("s_neg"))
        s_pe = ctx.enter_context(nc.semaphore("s_pe"))
        s_mag = ctx.enter_context(nc.semaphore("s_mag"))
        s_epi = ctx.enter_context(nc.semaphore("s_epi"))
        s_res = ctx.enter_context(nc.semaphore("s_res"))

        # Manual BassBlock: skip __exit__'s all-engine drain + EVSEM
        # butterfly (~10us of pure tail). All data hazards are sem-guarded
        # and every DMA's completion is waited on, so no end barrier needed.
        block = bass.BassBlock(nc, f"block_{nc.next_id()}")
        nc.cur_block = block

        @block.sync
        def _(sync):
            # x^T -> SBUF (512 KB, one descriptor set across 16 SDMA engines)
            sync.dma_start(out=xs[:, :, :], in_=xt3).then_inc(dma_x, 16)
            # result -> DRAM
            sync.wait_ge(s_res, 1)
            sync.dma_start(out=out[:, :], in_=res[:, :]).then_inc(dma_o, 16)
            sync.wait_ge(dma_o, 16)

        @block.scalar
        def _(scalar):
            # mask^T via scalar's HWDGE queue, in parallel with xt's
            scalar.dma_start(out=ms[:, :, :], in_=mt3).then_inc(dma_m, 16)
            scalar.wait_ge(s_sq, 1)
            nc.scalar.activation(ln[:, :, 0:B], sq[:, :, :], AF.Ln).then_inc(s_ln, 1)
            scalar.wait_ge(s_pe, 1)
            # exp(0.5 * S) ; S^T sits in ps[:, 0:B]
            nc.scalar.activation(mag[:, :], ps[:, 0:B], AF.Exp, scale=0.5).then_inc(
                s_mag, 1
            )

        @block.vector
        def _(vector):
            vector.wait_ge(dma_x, 16)
            # x^2 on DVE frees ACT for ln
            nc.vector.tensor_mul(sq[:, :, :], xs[:, :, :], xs[:, :, :]).then_inc(s_sq, 1)
            # neg indicator (x < 0) -> 1.0 / 0.0
            nc.vector.tensor_scalar(
                ln[:, :, B:2 * B], xs[:, :, :], 0.0, None, op0=ALU.is_lt
            ).then_inc(s_neg, 1)
            # epilogue: C^T (exact-integer negative count) sits in ps[:, B:2B].
            # res = mag XOR ((C & 1) << 31): parity flips the float sign bit.
            # Wait for ACT's Exp to finish its PSUM read first: concurrent
            # ACT-read + DVE-read of the same PSUM bank faults the exec unit.
            vector.wait_ge(s_mag, 1)
            nc.vector.tensor_copy(ci[:, :], ps[:, B:2 * B]).then_inc(s_epi, 1)
            vector.wait_ge(s_epi, 1)
            nc.vector.tensor_scalar(
                ci[:, :], ci[:, :], 31, None, op0=ALU.logical_shift_left
            ).then_inc(s_epi, 1)
            vector.wait_ge(s_epi, 2)
            vector.wait_ge(s_mag, 1)
            nc.vector.tensor_tensor(
                res[:, :].bitcast(I32), ci[:, :], mag[:, :].bitcast(I32),
                op=ALU.bitwise_xor,
            ).then_inc(s_res, 1)

        @block.tensor
        def _(tensor):
            tensor.wait_ge(dma_m, 16)
            tensor.wait_ge(s_ln, 1)
            tensor.wait_ge(s_neg, 1)
            for k in range(NK):
                mm = nc.tensor.matmul(
                    ps[:, :],
                    lhsT=ms[:, k, :],
                    rhs=ln[:, k, :],
                    start=(k == 0),
                    stop=(k == NK - 1),
                )
            mm.then_inc(s_pe, 1)

        # No end branches at all: each engine's instruction stream simply
        # ends inside its own body block (a br to a shared end_bb costs a
        # ~3-4us IRAM-miss stall per engine).
        nc.cur_block = None

    nc.finalize()
    return nc


_NC_CACHE = None


def _get_nc():
    global _NC_CACHE
    if _NC_CACHE is None:
        _NC_CACHE = build_nc()
    return _NC_CACHE


def _pack(aT: np.ndarray) -> np.ndarray:
    # [IN, W] -> [128, NK*W] SBUF image: row p = concat_k aT[k*128+p, :]
    w = aT.shape[1]
    return np.ascontiguousarray(
        aT.reshape(NK, 128, w).transpose(1, 0, 2).reshape(128, NK * w)
    )


def make_in_maps(x: np.ndarray, layer_mask: np.ndarray):
    xt = _pack(x.T.astype(np.float32))  # [128, NK*B]
    in_maps = []
    for c in range(NCORES):
        mt = _pack(
            layer_mask[c * O_SHARD:(c + 1) * O_SHARD, :].T.astype(np.float32)
        )  # [128, NK*O_SHARD]
        in_maps.append({"xt": xt, "mt": mt})
    return in_maps


def assemble_out(results):
    # results[c]["out"] is [O_SHARD, B] = out_full[:, shard].T
    return np.concatenate([r["out"].T for r in results], axis=1)


def run(x, layer_mask, trace=False, **kw):
    nc = _get_nc()
    in_maps = make_in_maps(np.asarray(x), np.asarray(layer_mask))
    res = run_bass_kernel_spmd(nc, in_maps, core_ids=list(range(NCORES)), trace=trace, **kw)
    return assemble_out(res.results), res


def kernel(x: np.ndarray, layer_mask: np.ndarray) -> np.ndarray:
    out, _ = run(x, layer_mask, trace=False)
    return out.astype(np.float32)


# revision 27
# speedup vs baseline: 1.0748x; 1.0748x over previous
"""Trainium2 Bass kernel for nn_CnUpdateLayer (segment_reduce / LDPC check-node update).

reference: out[b, i] = prod_{j : mask[i,j]==1} x[b, j]   (x ~ N(0,1), never exactly 0)

Log-domain trick turns the masked product into one dense matmul pass:
    S[b,i] = sum_j mask[i,j] * ln(x[b,j]^2)        -> magnitude = exp(0.5*S)
    C[b,i] = sum_j mask[i,j] * (x[b,j] < 0)        -> sign via parity of C
Both contractions share the stationary mask operand, so each K-tile runs as ONE
TensorEngine matmul with the moving operand [ln(x^2)^T | neg^T] (N=256,
float32r: full-rate fp32 matmul, fp32 PSUM accumulate).

Sharding: OUT columns (mask rows) x 8 cores, no collectives. Each core gets
x^T (replicated) and its mask shard pre-packed on host into the exact SBUF
image, so the contraction dim (IN) lands on SBUF partitions with no on-device
transpose and DMAs are fully contiguous per partition.

Raw Bass (no Tile): hand-placed semaphores, chunked DMA/compute pipeline,
no end-of-kernel barrier (all DMAs explicitly completion-waited).

Hardcoded problem shape: x [128, 1024] f32, layer_mask [1024, 1024] f32.
"""

import sys
from contextlib import ExitStack

import numpy as np

sys.path.insert(0, "/opt/trn_rl_repo")

import concourse.bacc as bacc
import concourse.bass as bass
from concourse import mybir
from concourse.bass_utils import run_bass_kernel_spmd

B = 128          # batch
IN = 1024        # in_features (contraction dim)
OUT = 1024       # out_features
NCORES = 8
O_SHARD = OUT // NCORES   # 128 mask rows per core
NK = IN // 128            # 8 K-tiles
NCH = 2                   # DMA/compute chunks (NK/NCH K-tiles per chunk)
KPC = NK // NCH           # K-tiles per chunk

F32 = mybir.dt.float32
F32R = mybir.dt.float32r
I32 = mybir.dt.int32
AF = mybir.ActivationFunctionType
ALU = mybir.AluOpType

# act_func_sets[6] = natural_log_exp_and_others: serves ln + exp (+ square/copy)
ACT_TABLE_LN_EXP = 6


def build_nc():
    nc = bacc.Bacc(None, target_bir_lowering=False)
    # Host pre-packs the exact SBUF image: row p holds [xT[k*128+p, :] for k in NK]
    xt = nc.declare_dram_parameter("xt", [128, NK * B], F32, isOutput=False)
    mt = nc.declare_dram_parameter("mt", [128, NK * O_SHARD], F32R, isOutput=False)
    out = nc.declare_dram_parameter("out", [O_SHARD, B], F32, isOutput=True)

    xt3 = xt[:, :].rearrange("p (k b) -> p k b", k=NK)       # [128, NK, B]
    mt3 = mt[:, :].rearrange("p (k o) -> p k o", k=NK)       # [128, NK, O]

    with ExitStack() as ctx:
        xs = ctx.enter_context(nc.sbuf_tensor([128, NK, B], F32))
        ms = ctx.enter_context(nc.sbuf_tensor([128, NK, O_SHARD], F32R))
        sq = ctx.enter_context(nc.sbuf_tensor([128, NK, B], F32))
        ln = ctx.enter_context(nc.sbuf_tensor([128, NK, 2 * B], F32R))
        ps = ctx.enter_context(nc.psum_tensor([128, 2 * B], F32))
        mag = ctx.enter_context(nc.sbuf_tensor([128, B], F32))
        ci = ctx.enter_context(nc.sbuf_tensor([128, B], I32))
        res = ctx.enter_context(nc.sbuf_tensor([128, B], F32))

        dma_x = [ctx.enter_context(nc.semaphore(f"dma_x{c}")) for c in range(NCH)]
        dma_m = [ctx.enter_context(nc.semaphore(f"dma_m{c}")) for c in range(NCH)]
        dma_o = ctx.enter_context(nc.semaphore("dma_o"))
        s_sq = ctx.enter_context(nc.semaphore("s_sq"))
        s_ln = ctx.enter_context(nc.semaphore("s_ln"))
        s_neg = ctx.enter_context(nc.semaphore("s_neg"))
        s_pe = ctx.enter_context(nc.semaphore("s_pe"))
        s_mag = ctx.enter_context(nc.semaphore("s_mag"))
        s_epi = ctx.enter_context(nc.semaphore("s_epi"))

        # Manual BassBlock: skip __exit__'s all-engine drain + EVSEM
        # butterfly. All hazards are sem-guarded and every DMA's completion
        # is waited on, so no end barrier is needed.
        block = bass.BassBlock(nc, f"block_{nc.next_id()}")
        nc.cur_block = block

        ksl = [slice(c * KPC, (c + 1) * KPC) for c in range(NCH)]

        @block.sync
        def _(sync):
            for c in range(NCH):
                sync.dma_start(out=xs[:, ksl[c], :], in_=xt3[:, ksl[c], :]).then_inc(
                    dma_x[c], 16
                )
            # result -> DRAM
            sync.wait_ge(s_epi, 3)
            sync.dma_start(out=out[:, :], in_=res[:, :]).then_inc(dma_o, 16)
            sync.wait_ge(dma_o, 16)

        @block.scalar
        def _(scalar):
            # mask chunks via scalar's HWDGE queue, in parallel with xt's
            for c in range(NCH):
                scalar.dma_start(out=ms[:, ksl[c], :], in_=mt3[:, ksl[c], :]).then_inc(
                    dma_m[c], 16
                )
            # preload the one act table that serves both Ln and Exp
            scalar.add_instruction(
                mybir.InstLoadActFuncSet(
                    name=nc.get_next_instruction_name(),
                    ins=[],
                    outs=[],
                    act_func_set_id=ACT_TABLE_LN_EXP,
                )
            )
            for c in range(NCH):
                scalar.wait_ge(s_sq, c + 1)
                nc.scalar.activation(
                    ln[:, ksl[c], 0:B], sq[:, ksl[c], :], AF.Ln
                ).then_inc(s_ln, 1)
            scalar.wait_ge(s_pe, 1)
            # exp(0.5 * S) ; S^T sits in ps[:, 0:B]
            nc.scalar.activation(mag[:, :], ps[:, 0:B], AF.Exp, scale=0.5).then_inc(
                s_mag, 1
            )

        @block.vector
        def _(vector):
            for c in range(NCH):
                vector.wait_ge(dma_x[c], 16)
                # x^2 on DVE frees ACT for ln
                nc.vector.tensor_mul(
                    sq[:, ksl[c], :], xs[:, ksl[c], :], xs[:, ksl[c], :]
                ).then_inc(s_sq, 1)
                # neg indicator (x < 0) -> 1.0 / 0.0
                nc.vector.tensor_scalar(
                    ln[:, ksl[c], B:2 * B], xs[:, ksl[c], :], 0.0, None, op0=ALU.is_lt
                ).then_inc(s_neg, 1)
            # epilogue: C^T (exact-integer negative count) sits in ps[:, B:2B].
            # res = mag XOR ((C & 1) << 31): parity flips the float sign bit.
            # ACT's Exp must finish its PSUM read first (concurrent ACT+DVE
            # reads of one PSUM bank fault the exec unit).
            vector.wait_ge(s_mag, 1)
            nc.vector.tensor_copy(ci[:, :], ps[:, B:2 * B]).then_inc(s_epi, 1)
            vector.wait_ge(s_epi, 1)
            nc.vector.tensor_scalar(
                ci[:, :], ci[:, :], 31, None, op0=ALU.logical_shift_left
            ).then_inc(s_epi, 1)
            vector.wait_ge(s_epi, 2)
            nc.vector.tensor_tensor(
                res[:, :].bitcast(I32), ci[:, :], mag[:, :].bitcast(I32),
                op=ALU.bitwise_xor,
            ).then_inc(s_epi, 1)

        @block.tensor
        def _(tensor):
            for c in range(NCH):
                tensor.wait_ge(dma_m[c], 16)
                tensor.wait_ge(s_ln, c + 1)
                tensor.wait_ge(s_neg, c + 1)
                for k in range(c * KPC, (c + 1) * KPC):
                    mm = nc.tensor.matmul(
                        ps[:, :],
                        lhsT=ms[:, k, :],
                        rhs=ln[:, k, :],
                        start=(k == 0),
                        stop=(k == NK - 1),
                    )
            mm.then_inc(s_pe, 1)

        # No end branches: each engine's stream simply ends in its own body
        # (a br to a shared end bb costs a ~3-4us IRAM-miss stall per engine).
        nc.cur_block = None

    nc.finalize()
    return nc


_NC_CACHE = None


def _get_nc():
    global _NC_CACHE
    if _NC_CACHE is None:
        _NC_CACHE = build_nc()
    return _NC_CACHE


def _pack(aT: np.ndarray) -> np.ndarray:
    # [IN, W] -> [128, NK*W] SBUF image: row p = concat_k aT[k*128+p, :]
    w = aT.shape[1]
    return np.ascontiguousarray(
        aT.reshape(NK, 128, w).transpose(1, 0, 2).reshape(128, NK * w)
    )


def make_in_maps(x: np.ndarray, layer_mask: np.ndarray):
    xt = _pack(x.T.astype(np.float32))  # [128, NK*B]
    in_maps = []
    for c in range(NCORES):
        mt = _pack(
            layer_mask[c * O_SHARD:(c + 1) * O_SHARD, :].T.astype(np.float32)
        )  # [128, NK*O_SHARD]
        in_maps.append({"xt": xt, "mt": mt})
    return in_maps


def assemble_out(results):
    # results[c]["out"] is [O_SHARD, B] = out_full[:, shard].T
    return np.concatenate([r["out"].T for r in results], axis=1)


def run(x, layer_mask, trace=False, **kw):
    nc = _get_nc()
    in_maps = make_in_maps(np.asarray(x), np.asarray(layer_mask))
    res = run_bass_kernel_spmd(nc, in_maps, core_ids=list(range(NCORES)), trace=trace, **kw)
    return assemble_out(res.results), res


def kernel(x: np.ndarray, layer_mask: np.ndarray) -> np.ndarray:
    out, _ = run(x, layer_mask, trace=False)
    return out.astype(np.float32)
